# revision 1
# baseline (speedup 1.0000x reference)
"""Trainium2 Bass kernel for nn_Eq_NLMP_18013092840057 (gnn_message_passing).

Strategy:
  * Host: sort edges by dst; shard into 8 contiguous node ranges (1280
    nodes/core) so no cross-core reduction is needed; group each core's
    edges into 10 windows of 128 destination nodes; pad every window to a
    uniform tile count (T_w tiles of 128 edges, dummy edges have norm=0).
    Host also precomputes the tiny radial-MLP hidden layers h1/h2 (emb @
    fc_w1, 10->16) and pre-scales/permutes the fc_w2 matrices.
  * Device (per core): PE generates the per-edge tensor-product weights
    (h1/h2 [16] x fc_w2 [16,896] per tile), indirect DMA gathers
    x[src]/x[dst], DVE/ACT evaluate the equivariant tensor products and
    gating, and an accumulating one-hot matmul in PSUM performs the
    per-window segment sum.  Window results stream out with plain DMAs.
"""
import sys
import numpy as np

try:
    import concourse.bass as bass  # noqa: F401
except Exception:  # pragma: no cover
    sys.path.insert(0, "/opt/trn_rl_repo")

import concourse.bass as bass
import concourse.bacc as bacc
import concourse.tile as tile
from concourse import mybir
from concourse.bass_utils import run_bass_kernel_spmd

SQRT3 = 3.0 ** 0.5
P = 128
NCORES = 8
dt = mybir.dt
Alu = mybir.AluOpType
Act = mybir.ActivationFunctionType

_KERNEL_CACHE = {}


# --------------------------------------------------------------------------
# Host-side preparation
# --------------------------------------------------------------------------

def _host_prep(x, edge_src, edge_dst, edge_vec, emb, norm,
               fc1_w1, fc1_w2, fc2_w1, fc2_w2):
    N = x.shape[0]
    E = edge_src.shape[0]
    npc = ((N + NCORES * P - 1) // (NCORES * P)) * P          # nodes per core
    wpc = npc // P                                             # windows/core

    order = np.argsort(edge_dst, kind="stable")
    dst_s = edge_dst[order]
    win = (dst_s // P).astype(np.int64)
    n_windows = NCORES * wpc
    counts = np.bincount(win, minlength=n_windows)
    t_w = int(max(1, ((counts + P - 1) // P).max()))
    nt = t_w * wpc                                             # tiles per core
    ep = nt * P                                                # padded edges/core

    # padded per-window edge-id table
    idx_pad = np.full((n_windows, t_w * P), -1, np.int64)
    starts = np.concatenate([[0], np.cumsum(counts)])
    for w in range(n_windows):
        c = counts[w]
        idx_pad[w, :c] = order[starts[w]:starts[w] + c]

    # host hidden layers (10 -> 16), relu
    h1 = np.maximum(emb @ fc1_w1 / np.sqrt(np.float32(10.0)), 0.0).astype(np.float32)
    h2 = np.maximum(emb @ fc2_w1 / np.sqrt(np.float32(10.0)), 0.0).astype(np.float32)

    # permuted / pre-scaled fc2 weights  [16, 896]
    s = np.float32(1.0 / np.sqrt(16.0))
    a1 = np.float32(1.0 / np.sqrt(32.0))
    a2 = np.float32(1.0 / np.sqrt(16.0))
    f1 = (fc1_w2 * (s * a1)).astype(np.float32)
    ss, vv, sv, vs = f1[:, 0:128], f1[:, 128:256], f1[:, 256:384], f1[:, 384:512]
    fc1p = np.concatenate([ss, sv * np.float32(SQRT3), vv, vs], axis=1)
    f2 = (fc2_w2 * (s * a2)).astype(np.float32)
    Ass, Avv = f2[:, 0:64], f2[:, 64:128]
    Bss, Bvv = f2[:, 128:192], f2[:, 192:256]
    Csv, Cvs = f2[:, 256:320], f2[:, 320:384]
    fc2p = np.concatenate([Ass, Bss, Csv * np.float32(SQRT3), Avv, Bvv, Cvs], axis=1)
    fcw = np.concatenate([fc1p, fc2p], axis=1)                 # [16, 896]

    def interleave(arr):
        # [ep, F] -> [128, nt*F]  with edge (t,p) at [p, t*F:(t+1)*F]
        F = arr.shape[1]
        return np.ascontiguousarray(
            arr.reshape(nt, P, F).transpose(1, 0, 2).reshape(P, nt * F))

    in_maps = []
    for m in range(NCORES):
        ids = idx_pad[m * wpc:(m + 1) * wpc].reshape(-1)       # [ep]
        valid = ids >= 0
        idc = np.where(valid, ids, 0)
        vm = valid[:, None]

        vec_c = np.where(vm, edge_vec[idc], np.float32(1.0)).astype(np.float32)
        norm_c = np.where(valid, norm[idc], 0.0).astype(np.float32)[:, None]
        src_c = np.where(valid, edge_src[idc], 0).astype(np.int32)[:, None]
        dstg_c = np.where(valid, edge_dst[idc], 0).astype(np.int32)[:, None]
        dstf_c = np.where(valid, edge_dst[idc] % P, 0).astype(np.float32)[:, None]
        h1_c = np.where(vm, h1[idc], 0.0).astype(np.float32)   # [ep,16]
        h2_c = np.where(vm, h2[idc], 0.0).astype(np.float32)

        in_maps.append({
            "x": np.ascontiguousarray(x.astype(np.float32)),
            "fcw": np.ascontiguousarray(fcw),
            "h1t": np.ascontiguousarray(h1_c.T),               # [16, ep]
            "h2t": np.ascontiguousarray(h2_c.T),
            "vecil": interleave(vec_c),
            "normil": interleave(norm_c),
            "srcil": interleave(src_c),
            "dstgil": interleave(dstg_c),
            "dstfil": interleave(dstf_c),
        })
    return in_maps, N, npc, wpc, t_w, nt, ep


# --------------------------------------------------------------------------
# Bass program
# --------------------------------------------------------------------------

def _build(N, npc, wpc, t_w, nt, ep):
    nc = bacc.Bacc("TRN2", target_bir_lowering=False)
    f32 = dt.float32

    x_d = nc.dram_tensor("x", [N, 32], f32, kind="ExternalInput")
    fcw_d = nc.dram_tensor("fcw", [16, 896], f32, kind="ExternalInput")
    h1t_d = nc.dram_tensor("h1t", [16, ep], f32, kind="ExternalInput")
    h2t_d = nc.dram_tensor("h2t", [16, ep], f32, kind="ExternalInput")
    vec_d = nc.dram_tensor("vecil", [P, nt * 3], f32, kind="ExternalInput")
    norm_d = nc.dram_tensor("normil", [P, nt], f32, kind="ExternalInput")
    src_d = nc.dram_tensor("srcil", [P, nt], dt.int32, kind="ExternalInput")
    dstg_d = nc.dram_tensor("dstgil", [P, nt], dt.int32, kind="ExternalInput")
    dstf_d = nc.dram_tensor("dstfil", [P, nt], f32, kind="ExternalInput")
    out_d = nc.dram_tensor("out", [npc, 32], f32, kind="ExternalOutput")

    with tile.TileContext(nc) as tc:
        with tc.tile_pool(name="const", bufs=1) as cpool, \
             tc.tile_pool(name="io", bufs=2) as io, \
             tc.tile_pool(name="big", bufs=1) as big, \
             tc.tile_pool(name="sm", bufs=1) as sm, \
             tc.tile_pool(name="oh", bufs=1) as ohp, \
             tc.tile_pool(name="wps", bufs=3, space="PSUM") as wps, \
             tc.tile_pool(name="aps", bufs=2, space="PSUM") as aps:

            fcw = cpool.tile([16, 896], f32)
            nc.sync.dma_start(fcw[:], fcw_d[:, :])
            iota_i = cpool.tile([P, P], dt.int32)
            iota_f = cpool.tile([P, P], f32)
            nc.gpsimd.iota(iota_i[:], pattern=[[1, P]], base=0, channel_multiplier=0)
            nc.vector.tensor_copy(iota_f[:], iota_i[:])

            # geometry prologue: r = vec/|vec| for every tile at once (one
            # Sqrt table residency; windows below then only need Tanh)
            vec_all = cpool.tile([P, nt, 3], f32)
            nc.sync.dma_start(vec_all[:], vec_d[:, :].rearrange("p (t k) -> p t k", k=3))
            rsq_a = sm.tile([P, nt, 3], f32, tag="tmpE")
            ssum_a = cpool.tile([P, nt], f32)
            rq_a = cpool.tile([P, nt], f32)
            nc.vector.tensor_tensor(out=rsq_a[:], in0=vec_all[:], in1=vec_all[:], op=Alu.mult)
            nc.vector.tensor_reduce(out=ssum_a[:], in_=rsq_a[:],
                                    axis=mybir.AxisListType.X, op=Alu.add)
            nc.scalar.activation(ssum_a[:], ssum_a[:], Act.Sqrt)
            nc.vector.reciprocal(rq_a[:], ssum_a[:])
            nc.vector.tensor_tensor(out=vec_all[:], in0=vec_all[:],
                                    in1=rq_a[:].unsqueeze(2).broadcast_to([P, nt, 3]),
                                    op=Alu.mult)

            for w in range(wpc):
                tb = w * t_w            # tile base
                eb = tb * P             # edge base

                h1c = io.tile([16, t_w * P], f32, tag="h1c")
                h2c = io.tile([16, t_w * P], f32, tag="h2c")
                nrm = io.tile([P, t_w], f32, tag="nrm")
                srci = io.tile([P, t_w], dt.int32, tag="srci")
                dstgi = io.tile([P, t_w], dt.int32, tag="dstgi")
                dstf = io.tile([P, t_w], f32, tag="dstf")
                nc.sync.dma_start(h1c[:], h1t_d[:, eb:eb + t_w * P])
                nc.sync.dma_start(h2c[:], h2t_d[:, eb:eb + t_w * P])
                nc.sync.dma_start(nrm[:], norm_d[:, tb:tb + t_w])
                nc.sync.dma_start(srci[:], src_d[:, tb:tb + t_w])
                nc.sync.dma_start(dstgi[:], dstg_d[:, tb:tb + t_w])
                nc.sync.dma_start(dstf[:], dstf_d[:, tb:tb + t_w])

                xs = io.tile([P, t_w, 32], f32, tag="xs")
                xd = io.tile([P, t_w, 32], f32, tag="xd")
                W1 = big.tile([P, t_w, 512], f32, tag="W1")
                W2 = big.tile([P, t_w, 384], f32, tag="W2")

                for t in range(t_w):
                    nc.gpsimd.indirect_dma_start(
                        out=xs[:, t, :], out_offset=None, in_=x_d[:, :],
                        in_offset=bass.IndirectOffsetOnAxis(ap=srci[:, t:t + 1], axis=0))
                    nc.gpsimd.indirect_dma_start(
                        out=xd[:, t, :], out_offset=None, in_=x_d[:, :],
                        in_offset=bass.IndirectOffsetOnAxis(ap=dstgi[:, t:t + 1], axis=0))
                    wp = wps.tile([P, 1024], f32, tag="wp")
                    nc.tensor.matmul(out=wp[:, 0:512],
                                     lhsT=h1c[:, t * P:(t + 1) * P],
                                     rhs=fcw[:, 0:512], start=True, stop=True)
                    nc.tensor.matmul(out=wp[:, 512:896],
                                     lhsT=h2c[:, t * P:(t + 1) * P],
                                     rhs=fcw[:, 512:896], start=True, stop=True)
                    nc.scalar.copy(W1[:, t, :], wp[:, 0:512])
                    nc.scalar.copy(W2[:, t, :], wp[:, 512:896])

                r = vec_all[:, tb:tb + t_w, :]

                # ---- vdot1[u] = sum_k v1[u,k] r[k] ---------------------------
                tmpE = sm.tile([P, t_w, 16, 3], f32, tag="tmpE")
                vdot1 = sm.tile([P, t_w, 16], f32, tag="vdot1")
                rb8 = r[:].unsqueeze(2).broadcast_to([P, t_w, 8, 3])
                xsv = xs[:, :, 8:32].rearrange("p t (u k) -> p t u k", u=8, k=3)
                xdv = xd[:, :, 8:32].rearrange("p t (u k) -> p t u k", u=8, k=3)
                nc.vector.tensor_tensor(out=tmpE[:, :, 0:8, :], in0=xsv, in1=rb8, op=Alu.mult)
                nc.vector.tensor_tensor(out=tmpE[:, :, 8:16, :], in0=xdv, in1=rb8, op=Alu.mult)
                nc.vector.tensor_reduce(out=vdot1[:], in_=tmpE[:],
                                        axis=mybir.AxisListType.X, op=Alu.add)

                # ---- TP1 scalar paths: tmpA [p,t,3,16,8] ---------------------
                # (DVE ISA allows at most 3 free dims per AP: split by path,
                #  merge contiguous (u,w) for the reduction tree.)
                tmpA = big.tile([P, t_w, 3, 16, 8], f32, tag="tmpA")
                s1s = xs[:, :, 0:8].unsqueeze(3).broadcast_to([P, t_w, 8, 8])
                s1d = xd[:, :, 0:8].unsqueeze(3).broadcast_to([P, t_w, 8, 8])
                for q in (0, 1):   # 0: Wss, 1: Wsv
                    Wq = W1[:, :, q * 128:(q + 1) * 128].rearrange(
                        "p t (u w) -> p t u w", u=16, w=8)
                    nc.vector.tensor_tensor(out=tmpA[:, :, q, 0:8, :],
                                            in0=Wq[:, :, 0:8, :], in1=s1s, op=Alu.mult)
                    nc.vector.tensor_tensor(out=tmpA[:, :, q, 8:16, :],
                                            in0=Wq[:, :, 8:16, :], in1=s1d, op=Alu.mult)
                Wvv = W1[:, :, 256:384].rearrange("p t (u w) -> p t u w", u=16, w=8)
                vdb = vdot1[:].unsqueeze(3).broadcast_to([P, t_w, 16, 8])
                nc.vector.tensor_tensor(out=tmpA[:, :, 2, :, :], in0=Wvv, in1=vdb, op=Alu.mult)
                tmpAm = tmpA[:].rearrange("p t q u w -> p t q (u w)")
                k = 8
                while k >= 1:
                    nc.vector.tensor_tensor(out=tmpAm[:, :, :, 0:k * 8],
                                            in0=tmpAm[:, :, :, 0:k * 8],
                                            in1=tmpAm[:, :, :, k * 8:2 * k * 8], op=Alu.add)
                    k //= 2
                st = sm.tile([P, t_w, 8], f32, tag="st")
                nc.vector.tensor_tensor(out=st[:], in0=tmpA[:, :, 0, 0, :],
                                        in1=tmpA[:, :, 2, 0, :], op=Alu.add)
                # S_sv (sqrt3-scaled) lives at tmpA[:, :, 1, 0, :]

                # ---- TP1 vs path: tmpB [p,t,16,8,3] --------------------------
                tmpB = big.tile([P, t_w, 16, 8, 3], f32, tag="tmpB")
                Wvs = W1[:, :, 384:512].rearrange("p t (u w) -> p t u w", u=16, w=8)
                for kk in range(3):
                    nc.vector.tensor_tensor(
                        out=tmpB[:, :, 0:8, :, kk],
                        in0=Wvs[:, :, 0:8, :],
                        in1=xsv[:, :, :, kk].unsqueeze(3).broadcast_to([P, t_w, 8, 8]),
                        op=Alu.mult)
                    nc.vector.tensor_tensor(
                        out=tmpB[:, :, 8:16, :, kk],
                        in0=Wvs[:, :, 8:16, :],
                        in1=xdv[:, :, :, kk].unsqueeze(3).broadcast_to([P, t_w, 8, 8]),
                        op=Alu.mult)
                tmpBm = tmpB[:].rearrange("p t u w k -> p t u (w k)")
                k = 8
                while k >= 1:
                    nc.vector.tensor_tensor(out=tmpBm[:, :, 0:k, :],
                                            in0=tmpBm[:, :, 0:k, :],
                                            in1=tmpBm[:, :, k:2 * k, :], op=Alu.add)
                    k //= 2

                # ---- v_t = S_sv*r + V_vs ; vdot2 = sum_k v_t r ---------------
                v_t = sm.tile([P, t_w, 8, 3], f32, tag="v_t")
                rbw = r[:].unsqueeze(2).broadcast_to([P, t_w, 8, 3])
                nc.vector.tensor_tensor(
                    out=v_t[:],
                    in0=tmpA[:, :, 1, 0, :].unsqueeze(3).broadcast_to([P, t_w, 8, 3]),
                    in1=rbw, op=Alu.mult)
                nc.vector.tensor_tensor(out=v_t[:], in0=v_t[:],
                                        in1=tmpB[:, :, 0, :, :], op=Alu.add)
                tmpD = sm.tile([P, t_w, 8, 3], f32, tag="tmpD")
                vdot2 = sm.tile([P, t_w, 8], f32, tag="vdot2")
                nc.vector.tensor_tensor(out=tmpD[:], in0=v_t[:], in1=rbw, op=Alu.mult)
                nc.vector.tensor_reduce(out=vdot2[:], in_=tmpD[:],
                                        axis=mybir.AxisListType.X, op=Alu.add)

                # ---- TP2 scalar paths ---------------------------------------
                tmpF = big.tile([P, t_w, 3, 8, 8], f32, tag="tmpF")
                tmpG = big.tile([P, t_w, 2, 8, 8], f32, tag="tmpE")
                stb = st[:].unsqueeze(3).broadcast_to([P, t_w, 8, 8])
                vd2b = vdot2[:].unsqueeze(3).broadcast_to([P, t_w, 8, 8])
                for q in range(3):
                    WFq = W2[:, :, q * 64:(q + 1) * 64].rearrange(
                        "p t (u w) -> p t u w", u=8, w=8)
                    nc.vector.tensor_tensor(out=tmpF[:, :, q, :, :], in0=WFq,
                                            in1=stb, op=Alu.mult)
                for q in range(2):
                    WGq = W2[:, :, 192 + q * 64:192 + (q + 1) * 64].rearrange(
                        "p t (u w) -> p t u w", u=8, w=8)
                    nc.vector.tensor_tensor(out=tmpG[:, :, q, :, :], in0=WGq,
                                            in1=vd2b, op=Alu.mult)
                tmpFm = tmpF[:].rearrange("p t q u w -> p t q (u w)")
                tmpGm = tmpG[:].rearrange("p t q u w -> p t q (u w)")
                k = 4
                while k >= 1:
                    nc.vector.tensor_tensor(out=tmpFm[:, :, :, 0:k * 8],
                                            in0=tmpFm[:, :, :, 0:k * 8],
                                            in1=tmpFm[:, :, :, k * 8:2 * k * 8], op=Alu.add)
                    nc.vector.tensor_tensor(out=tmpGm[:, :, :, 0:k * 8],
                                            in0=tmpGm[:, :, :, 0:k * 8],
                                            in1=tmpGm[:, :, :, k * 8:2 * k * 8], op=Alu.add)
                    k //= 2
                sg = sm.tile([P, t_w, 2, 8], f32, tag="sg")
                nc.vector.tensor_tensor(out=sg[:, :, 0, :], in0=tmpF[:, :, 0, 0, :],
                                        in1=tmpG[:, :, 0, 0, :], op=Alu.add)
                nc.vector.tensor_tensor(out=sg[:, :, 1, :], in0=tmpF[:, :, 1, 0, :],
                                        in1=tmpG[:, :, 1, 0, :], op=Alu.add)

                # ---- TP2 vs path (Cvs) --------------------------------------
                tmpH = big.tile([P, t_w, 8, 8, 3], f32, tag="tmpA")
                Wcvs = W2[:, :, 320:384].rearrange("p t (u w) -> p t u w", u=8, w=8)
                for kk in range(3):
                    nc.vector.tensor_tensor(
                        out=tmpH[:, :, :, :, kk],
                        in0=Wcvs,
                        in1=v_t[:, :, :, kk].unsqueeze(3).broadcast_to([P, t_w, 8, 8]),
                        op=Alu.mult)
                tmpHm = tmpH[:].rearrange("p t u w k -> p t u (w k)")
                k = 4
                while k >= 1:
                    nc.vector.tensor_tensor(out=tmpHm[:, :, 0:k, :],
                                            in0=tmpHm[:, :, 0:k, :],
                                            in1=tmpHm[:, :, k:2 * k, :], op=Alu.add)
                    k //= 2

                # ---- gate + norm + edge_out ---------------------------------
                tsg = sm.tile([P, t_w, 2, 8], f32, tag="tsg")
                nc.scalar.activation(tsg[:], sg[:], Act.Tanh)
                vecs = sm.tile([P, t_w, 8, 3], f32, tag="vecs")
                nc.vector.tensor_tensor(
                    out=vecs[:],
                    in0=tmpF[:, :, 2, 0, :].unsqueeze(3).broadcast_to([P, t_w, 8, 3]),
                    in1=rbw, op=Alu.mult)
                nc.vector.tensor_tensor(out=vecs[:], in0=vecs[:],
                                        in1=tmpH[:, :, 0, :, :], op=Alu.add)
                # tgn = tanh(gates)*norm (folds the norm scale of the vector
                # block into the gate); tsn = tanh(scal)*norm
                tgn = sm.tile([P, t_w, 2, 8], f32, tag="tgn")
                nc.vector.tensor_tensor(
                    out=tgn[:], in0=tsg[:],
                    in1=nrm[:].unsqueeze(2).unsqueeze(3).broadcast_to([P, t_w, 2, 8]),
                    op=Alu.mult)
                eo = sm.tile([P, t_w, 32], f32, tag="eo")
                nc.vector.tensor_copy(eo[:, :, 0:8], tgn[:, :, 0, :])
                nc.vector.tensor_tensor(
                    out=eo[:, :, 8:32].rearrange("p t (w k) -> p t w k", w=8, k=3),
                    in0=vecs[:],
                    in1=tgn[:, :, 1, :].unsqueeze(3).broadcast_to([P, t_w, 8, 3]),
                    op=Alu.mult)

                # ---- windowed segment-sum via accumulating matmul -----------
                oht = ohp.tile([P, t_w, P], f32, tag="oht")
                nc.vector.tensor_tensor(
                    out=oht[:],
                    in0=dstf[:].unsqueeze(2).broadcast_to([P, t_w, P]),
                    in1=iota_f[:].unsqueeze(1).broadcast_to([P, t_w, P]),
                    op=Alu.is_equal)
                acc = aps.tile([P, 32], f32, tag="acc")
                for t in range(t_w):
                    nc.tensor.matmul(out=acc[:], lhsT=oht[:, t, :], rhs=eo[:, t, :],
                                     start=(t == 0), stop=(t == t_w - 1),
                                     skip_group_check=True)
                osb = sm.tile([P, 32], f32, tag="osb")
                nc.scalar.copy(osb[:], acc[:])
                nc.sync.dma_start(out_d[w * P:(w + 1) * P, :], osb[:])
    nc.compile()
    return nc


def _get_nc(key):
    if key not in _KERNEL_CACHE:
        _KERNEL_CACHE[key] = _build(*key)
    return _KERNEL_CACHE[key]


# --------------------------------------------------------------------------
# Entry point
# --------------------------------------------------------------------------

def kernel(x, edge_src, edge_dst, edge_vec, emb, norm, num_nodes,
           fc1_w1, fc1_w2, fc2_w1, fc2_w2, _trace=False):
    x = np.asarray(x, np.float32)
    edge_src = np.asarray(edge_src).astype(np.int64)
    edge_dst = np.asarray(edge_dst).astype(np.int64)
    edge_vec = np.asarray(edge_vec, np.float32)
    emb = np.asarray(emb, np.float32)
    norm = np.asarray(norm, np.float32)
    fc1_w1 = np.asarray(fc1_w1, np.float32)
    fc1_w2 = np.asarray(fc1_w2, np.float32)
    fc2_w1 = np.asarray(fc2_w1, np.float32)
    fc2_w2 = np.asarray(fc2_w2, np.float32)
    N = x.shape[0]
    assert int(num_nodes) == N

    in_maps, N, npc, wpc, t_w, nt, ep = _host_prep(
        x, edge_src, edge_dst, edge_vec, emb, norm,
        fc1_w1, fc1_w2, fc2_w1, fc2_w2)
    nc = _get_nc((N, npc, wpc, t_w, nt, ep))
    res = run_bass_kernel_spmd(nc, in_maps, core_ids=list(range(NCORES)),
                               trace=_trace)
    out = np.concatenate([res.results[m]["out"] for m in range(NCORES)], axis=0)
    if _trace:
        return out[:N].astype(np.float32), res
    return out[:N].astype(np.float32)



# revision 11
# speedup vs baseline: 3.1306x; 3.1306x over previous
"""Trainium2 Bass kernel for nn_Eq_NLMP_18013092840057 (gnn_message_passing).

Strategy (v2):
  * Host: equal edge split across 8 cores (20000 edges + pad to 20480).
    Host pre-gathers x[src]/x[dst], precomputes the radial-MLP hidden
    layers h1/h2 (10->16), r-hat, vdot1, and lays everything out
    edge-major bf16 ([128, nt, F], edge i at partition i%128, tile
    i//128).  The fc2 weight matrices are permuted to (w-outer, u-inner)
    blocks and scaled so all path norms fold in.
  * Device (per core): PE generates per-edge TP weights in bf16
    (h-tile [16,128] x fcw [16,896] per tile of 128 edges); ACT
    evacuates PSUM->SBUF bf16; DVE + GPSIMD split the equivariant
    tensor-product mults and reduction trees (all bf16, 2x packed mode);
    tanh gating on ACT; a single dma_scatter_add accumulates edge
    outputs into a per-core [N, 64] HBM buffer.
  * Host: sums the 8 per-core partials and reorders vector columns.
"""
import sys
import numpy as np

try:
    import concourse.bass as bass  # noqa: F401
except Exception:  # pragma: no cover
    sys.path.insert(0, "/opt/trn_rl_repo")

import concourse.bass as bass
import concourse.bacc as bacc
import concourse.tile as tile
from concourse import mybir
from concourse.bass_utils import run_bass_kernel_spmd

SQRT3 = np.float32(3.0 ** 0.5)
P = 128
NCORES = 8
G = 16               # tiles per group
dt = mybir.dt
Alu = mybir.AluOpType
Act = mybir.ActivationFunctionType

_KERNEL_CACHE = {}


# --------------------------------------------------------------------------
# Host-side preparation
# --------------------------------------------------------------------------

def _prep_fcw(fc1_w2, fc2_w2):
    """Permute + scale the 16->512 / 16->384 weight mats into one [16, 896]
    block layout: [ssvv(256) | sv(128) | vs(128) | AB(256) | Csv(64) | Cvs(64)],
    every block (w-outer, u-inner)."""
    a1 = np.float32(1.0 / np.sqrt(32.0))
    a2 = np.float32(1.0 / np.sqrt(16.0))
    s = np.float32(1.0 / np.sqrt(16.0))       # MLP second-layer 1/sqrt(fan_in)
    f1 = (fc1_w2 * s).astype(np.float32)      # [16, 512]
    f2 = (fc2_w2 * s).astype(np.float32)      # [16, 384]
    cols = np.zeros((16, 896), np.float32)
    # ssvv block: cols 0:256, c = w*32 + u; u<16 -> Wss (f1[:, u*8+w]) * a1,
    # u>=16 -> Wvv (f1[:, 128 + (u-16)*8 + w]) * a1
    for w in range(8):
        for u in range(32):
            src = u * 8 + w if u < 16 else 128 + (u - 16) * 8 + w
            cols[:, w * 32 + u] = f1[:, src] * a1
    # sv block: cols 256:384, c = 256 + w*16 + u ; f1[:, 256 + u*8+w] * a1*sqrt3
    for w in range(8):
        for u in range(16):
            cols[:, 256 + w * 16 + u] = f1[:, 256 + u * 8 + w] * (a1 * SQRT3)
    # vs block: cols 384:512 ; f1[:, 384 + u*8+w] * a1
    for w in range(8):
        for u in range(16):
            cols[:, 384 + w * 16 + u] = f1[:, 384 + u * 8 + w] * a1
    # AB block: cols 512:768. q=0 (A): u<8 Ass f2[:, u*8+w], u>=8 Avv
    # f2[:, 64 + (u-8)*8 + w]; q=1 (B): Bss 128+, Bvv 192+. all * a2
    for q in range(2):
        for w in range(8):
            for u in range(16):
                base = (0 if u < 8 else 64) + q * 128
                src = base + (u % 8) * 8 + w
                cols[:, 512 + q * 128 + w * 16 + u] = f2[:, src] * a2
    # Csv: cols 768:832 ; f2[:, 256 + u*8+w] * a2*sqrt3
    for w in range(8):
        for u in range(8):
            cols[:, 768 + w * 8 + u] = f2[:, 256 + u * 8 + w] * (a2 * SQRT3)
    # Cvs: cols 832:896 ; f2[:, 320 + u*8+w] * a2
    for w in range(8):
        for u in range(8):
            cols[:, 832 + w * 8 + u] = f2[:, 320 + u * 8 + w] * a2
    return cols


def _host_prep(x, edge_src, edge_dst, edge_vec, emb, norm,
               fc1_w1, fc1_w2, fc2_w1, fc2_w2):
    import ml_dtypes
    bf16 = ml_dtypes.bfloat16
    N = x.shape[0]
    E = edge_src.shape[0]
    epc = E // NCORES                          # edges per core (true)
    nt = ((epc + G * P - 1) // (G * P)) * G    # tiles per core (mult of G)
    ep = nt * P                                # padded edges per core

    fcw = _prep_fcw(fc1_w2, fc2_w2)
    h1 = np.maximum(emb @ fc1_w1 / np.sqrt(np.float32(10.0)), 0.0).astype(np.float32)
    h2 = np.maximum(emb @ fc2_w1 / np.sqrt(np.float32(10.0)), 0.0).astype(np.float32)
    rhat = (edge_vec / np.linalg.norm(edge_vec, axis=1, keepdims=True)).astype(np.float32)

    xs = x[edge_src]                           # [E, 32]
    xd = x[edge_dst]
    s1 = np.concatenate([xs[:, :8], xd[:, :8]], axis=1)          # [E,16]
    v1 = np.concatenate([xs[:, 8:].reshape(E, 8, 3),
                         xd[:, 8:].reshape(E, 8, 3)], axis=1)    # [E,16,3]
    vdot1 = np.einsum('euk,ek->eu', v1, rhat)                    # [E,16]
    f1 = np.concatenate([s1, vdot1], axis=1)                     # [E,32]
    v1k = np.ascontiguousarray(v1.transpose(0, 2, 1))            # [E,3,16]
    rr8 = np.repeat(rhat[:, :, None], 8, axis=2)                 # [E,3,8]
    n8 = np.repeat(norm[:, None], 8, axis=1)                     # [E,8]

    def interleave(arr, m):
        # core m's slice -> [P, nt, F]; edge i at [i%P, i//P]
        F = int(np.prod(arr.shape[1:])) if arr.ndim > 1 else 1
        a = arr[m * epc:(m + 1) * epc].reshape(epc, F)
        a = np.concatenate([a, np.zeros((ep - epc, F), a.dtype)], axis=0)
        return np.ascontiguousarray(a.reshape(nt, P, F).transpose(1, 0, 2)).astype(bf16)

    in_maps = []
    for m in range(NCORES):
        h1t = np.zeros((16, ep), np.float32)
        h2t = np.zeros((16, ep), np.float32)
        h1t[:, :epc] = h1[m * epc:(m + 1) * epc].T
        h2t[:, :epc] = h2[m * epc:(m + 1) * epc].T

        dst_c = edge_dst[m * epc:(m + 1) * epc].astype(np.int16)
        idx = np.zeros((P, ep // 16), np.int16)
        ii = np.arange(epc)
        idx[ii % 16, ii // 16] = dst_c

        in_maps.append({
            "fcw": fcw.astype(bf16),
            "h1t": h1t.astype(bf16),
            "h2t": h2t.astype(bf16),
            "f1": interleave(f1, m),
            "v1k": interleave(v1k, m),
            "rr8": interleave(rr8, m),
            "n8": interleave(n8, m),
            "sidx": idx,
            "zero64": np.zeros((N, 64), np.float32),
        })
    return in_maps, N, 0, 0, 0, nt, ep


# --------------------------------------------------------------------------
# Bass program
# --------------------------------------------------------------------------

def _build(N, npc, wpc, t_w, nt, ep):
    nc = bacc.Bacc("TRN2", target_bir_lowering=False)
    f32, bf16 = dt.float32, dt.bfloat16
    ng = nt // G

    fcw_d = nc.dram_tensor("fcw", [16, 896], bf16, kind="ExternalInput")
    h1t_d = nc.dram_tensor("h1t", [16, ep], bf16, kind="ExternalInput")
    h2t_d = nc.dram_tensor("h2t", [16, ep], bf16, kind="ExternalInput")
    f1_d = nc.dram_tensor("f1", [P, nt, 32], bf16, kind="ExternalInput")
    v1k_d = nc.dram_tensor("v1k", [P, nt, 48], bf16, kind="ExternalInput")
    rr8_d = nc.dram_tensor("rr8", [P, nt, 24], bf16, kind="ExternalInput")
    n8_d = nc.dram_tensor("n8", [P, nt, 8], bf16, kind="ExternalInput")
    sidx_d = nc.dram_tensor("sidx", [P, ep // 16], dt.int16, kind="ExternalInput")
    zero_d = nc.dram_tensor("zero64", [N, 64], f32, kind="ExternalInput")
    out_d = nc.dram_tensor("out", [N, 64], f32, kind="ExternalOutput")

    with tile.TileContext(nc) as tc:
        with tc.tile_pool(name="const", bufs=1) as cpool, \
             tc.tile_pool(name="io", bufs=2) as io, \
             tc.tile_pool(name="wsb", bufs=2) as wsb, \
             tc.tile_pool(name="mm", bufs=2) as mm, \
             tc.tile_pool(name="sm", bufs=2) as sm, \
             tc.tile_pool(name="eop", bufs=1) as eop, \
             tc.tile_pool(name="wps", bufs=2, space="PSUM") as wps:

            fcw = cpool.tile([16, 896], bf16)
            nc.sync.dma_start(fcw[:], fcw_d[:, :])
            sidx = cpool.tile([P, ep // 16], dt.int16)
            nc.sync.dma_start(sidx[:], sidx_d[:, :])
            eo = eop.tile([P, nt, 32], f32)

            # zero the output accumulator up front; the first scatter-add
            # fires ~2 full compute groups (tens of us) later
            nc.sync.dma_start(out_d[:, :], zero_d[:, :])

            for g in range(ng):
                tb = g * G

                h1g = io.tile([16, G * P], bf16, tag="h1g")
                h2g = io.tile([16, G * P], bf16, tag="h2g")
                f1g = io.tile([P, G, 32], bf16, tag="f1")
                v1g = io.tile([P, G, 3, 16], bf16, tag="v1")
                rr8g = io.tile([P, G, 3, 8], bf16, tag="rr8")
                n8g = io.tile([P, G, 8], bf16, tag="n8")
                nc.sync.dma_start(h1g[:], h1t_d[:, tb * P:(tb + G) * P])
                nc.sync.dma_start(h2g[:], h2t_d[:, tb * P:(tb + G) * P])
                nc.sync.dma_start(f1g[:], f1_d[:, tb:tb + G, :].rearrange(
                    "p t f -> p t f"))
                nc.sync.dma_start(v1g[:], v1k_d[:, tb:tb + G, :].rearrange(
                    "p t (k u) -> p t k u", k=3))
                nc.sync.dma_start(rr8g[:], rr8_d[:, tb:tb + G, :].rearrange(
                    "p t (k w) -> p t k w", k=3))
                nc.sync.dma_start(n8g[:], n8_d[:, tb:tb + G, :])

                # ---- PE weight-gen + ACT evacuation ----------------------
                W = wsb.tile([P, G, 896], bf16, tag="W")
                for q in range(G // 2):
                    wp = wps.tile([P, 2, 1024], f32, tag="wp")
                    for i in range(2):
                        t = 2 * q + i
                        nc.tensor.matmul(out=wp[:, i, 0:512],
                                         lhsT=h1g[:, t * P:(t + 1) * P],
                                         rhs=fcw[:, 0:512], start=True, stop=True)
                        nc.tensor.matmul(out=wp[:, i, 512:896],
                                         lhsT=h2g[:, t * P:(t + 1) * P],
                                         rhs=fcw[:, 512:896], start=True, stop=True)
                    nc.scalar.copy(W[:, 2 * q:2 * q + 2, :], wp[:, :, 0:896])

                # ---- TP1 ss+vv path (DVE) --------------------------------
                m32 = mm.tile([P, G, 8, 32], bf16, tag="m32")
                nc.vector.tensor_tensor(
                    out=m32[:],
                    in0=W[:, :, 0:256].rearrange("p g (w u) -> p g w u", w=8),
                    in1=f1g[:].unsqueeze(2).broadcast_to([P, G, 8, 32]),
                    op=Alu.mult)
                m32v = m32[:].rearrange("p g w u -> p (g w) u")
                for k in (16, 8, 4, 2):
                    nc.vector.tensor_tensor(out=m32v[:, :, 0:k], in0=m32v[:, :, 0:k],
                                            in1=m32v[:, :, k:2 * k], op=Alu.add)
                f2 = sm.tile([P, G, 16], bf16, tag="f2")
                nc.vector.tensor_tensor(out=f2[:, :, 0:8], in0=m32[:, :, :, 0],
                                        in1=m32[:, :, :, 1], op=Alu.add)

                # ---- TP1 sv path (DVE) -----------------------------------
                msv = mm.tile([P, G, 8, 16], bf16, tag="msv")
                nc.vector.tensor_tensor(
                    out=msv[:],
                    in0=W[:, :, 256:384].rearrange("p g (w u) -> p g w u", w=8),
                    in1=f1g[:, :, 0:16].unsqueeze(2).broadcast_to([P, G, 8, 16]),
                    op=Alu.mult)
                msvv = msv[:].rearrange("p g w u -> p (g w) u")
                for k in (8, 4, 2):
                    nc.vector.tensor_tensor(out=msvv[:, :, 0:k], in0=msvv[:, :, 0:k],
                                            in1=msvv[:, :, k:2 * k], op=Alu.add)
                ssv = sm.tile([P, G, 8], bf16, tag="ssv")
                nc.vector.tensor_tensor(out=ssv[:], in0=msv[:, :, :, 0],
                                        in1=msv[:, :, :, 1], op=Alu.add)

                # ---- TP1 vs path (Pool) ----------------------------------
                m16p = mm.tile([P, G, 3, 8, 16], bf16, tag="m16p")
                for k in range(3):
                    nc.gpsimd.tensor_tensor(
                        out=m16p[:, :, k, :, :],
                        in0=W[:, :, 384:512].rearrange("p g (w u) -> p g w u", w=8),
                        in1=v1g[:, :, k, :].unsqueeze(2).broadcast_to([P, G, 8, 16]),
                        op=Alu.mult)
                m16pv = m16p[:].rearrange("p g k w u -> p (g k w) u")
                for k in (8, 4, 2):
                    nc.gpsimd.tensor_tensor(out=m16pv[:, :, 0:k], in0=m16pv[:, :, 0:k],
                                            in1=m16pv[:, :, k:2 * k], op=Alu.add)
                vts = sm.tile([P, G, 3, 8], bf16, tag="vts")
                nc.gpsimd.tensor_tensor(out=vts[:], in0=m16p[:, :, :, :, 0],
                                        in1=m16p[:, :, :, :, 1], op=Alu.add)

                # ---- v_t, vdot2 (DVE) ------------------------------------
                v_t = sm.tile([P, G, 3, 8], bf16, tag="v_t")
                nc.vector.tensor_tensor(
                    out=v_t[:],
                    in0=ssv[:].unsqueeze(2).broadcast_to([P, G, 3, 8]),
                    in1=rr8g[:], op=Alu.mult)
                nc.vector.tensor_tensor(out=v_t[:], in0=v_t[:], in1=vts[:], op=Alu.add)
                vd3 = sm.tile([P, G, 3, 8], bf16, tag="vd3")
                nc.vector.tensor_tensor(out=vd3[:], in0=v_t[:], in1=rr8g[:], op=Alu.mult)
                vd2 = sm.tile([P, G, 8], bf16, tag="vd2")
                nc.vector.tensor_tensor(out=vd2[:], in0=vd3[:, :, 0, :],
                                        in1=vd3[:, :, 1, :], op=Alu.add)
                nc.vector.tensor_tensor(out=f2[:, :, 8:16], in0=vd2[:],
                                        in1=vd3[:, :, 2, :], op=Alu.add)

                # ---- TP2 A+B paths (DVE) ---------------------------------
                mab = mm.tile([P, G, 16, 16], bf16, tag="mab")
                nc.vector.tensor_tensor(
                    out=mab[:],
                    in0=W[:, :, 512:768].rearrange("p g (w u) -> p g w u", w=16),
                    in1=f2[:].unsqueeze(2).broadcast_to([P, G, 16, 16]),
                    op=Alu.mult)
                mabv = mab[:].rearrange("p g w u -> p (g w) u")
                for k in (8, 4, 2):
                    nc.vector.tensor_tensor(out=mabv[:, :, 0:k], in0=mabv[:, :, 0:k],
                                            in1=mabv[:, :, k:2 * k], op=Alu.add)
                sg = sm.tile([P, G, 2, 8], bf16, tag="sg")
                nc.vector.tensor_tensor(
                    out=sg[:], in0=mab[:, :, :, 0].rearrange("p g (q w) -> p g q w", q=2),
                    in1=mab[:, :, :, 1].rearrange("p g (q w) -> p g q w", q=2),
                    op=Alu.add)

                # ---- TP2 Csv path (DVE) ----------------------------------
                mcs = mm.tile([P, G, 8, 8], bf16, tag="mcs")
                nc.vector.tensor_tensor(
                    out=mcs[:],
                    in0=W[:, :, 768:832].rearrange("p g (w u) -> p g w u", w=8),
                    in1=f2[:, :, 0:8].unsqueeze(2).broadcast_to([P, G, 8, 8]),
                    op=Alu.mult)
                mcsv = mcs[:].rearrange("p g w u -> p (g w) u")
                for k in (4, 2):
                    nc.vector.tensor_tensor(out=mcsv[:, :, 0:k], in0=mcsv[:, :, 0:k],
                                            in1=mcsv[:, :, k:2 * k], op=Alu.add)
                scs = sm.tile([P, G, 8], bf16, tag="scs")
                nc.vector.tensor_tensor(out=scs[:], in0=mcs[:, :, :, 0],
                                        in1=mcs[:, :, :, 1], op=Alu.add)

                # ---- TP2 Cvs path (DVE) ----------------------------------
                mcv = mm.tile([P, G, 3, 8, 8], bf16, tag="mcv")
                for k in range(3):
                    nc.vector.tensor_tensor(
                        out=mcv[:, :, k, :, :],
                        in0=W[:, :, 832:896].rearrange("p g (w u) -> p g w u", w=8),
                        in1=v_t[:, :, k, :].unsqueeze(2).broadcast_to([P, G, 8, 8]),
                        op=Alu.mult)
                mcvv = mcv[:].rearrange("p g k w u -> p (g k w) u")
                for k in (4, 2):
                    nc.vector.tensor_tensor(out=mcvv[:, :, 0:k], in0=mcvv[:, :, 0:k],
                                            in1=mcvv[:, :, k:2 * k], op=Alu.add)
                vcs = sm.tile([P, G, 3, 8], bf16, tag="vcs")
                nc.vector.tensor_tensor(out=vcs[:], in0=mcv[:, :, :, :, 0],
                                        in1=mcv[:, :, :, :, 1], op=Alu.add)

                # ---- gate + pack ----------------------------------------
                vecs = sm.tile([P, G, 3, 8], bf16, tag="vecs")
                nc.vector.tensor_tensor(
                    out=vecs[:],
                    in0=scs[:].unsqueeze(2).broadcast_to([P, G, 3, 8]),
                    in1=rr8g[:], op=Alu.mult)
                nc.vector.tensor_tensor(out=vecs[:], in0=vecs[:], in1=vcs[:], op=Alu.add)
                tsg = sm.tile([P, G, 2, 8], bf16, tag="tsg")
                nc.scalar.activation(tsg[:], sg[:], Act.Tanh)
                tgn = sm.tile([P, G, 8], bf16, tag="tgn")
                nc.vector.tensor_tensor(out=tgn[:], in0=tsg[:, :, 1, :],
                                        in1=n8g[:], op=Alu.mult)
                nc.vector.tensor_tensor(out=eo[:, tb:tb + G, 0:8],
                                        in0=tsg[:, :, 0, :], in1=n8g[:], op=Alu.mult)
                nc.vector.tensor_tensor(
                    out=eo[:, tb:tb + G, 8:32].rearrange("p g (k w) -> p g k w", k=3),
                    in0=vecs[:],
                    in1=tgn[:].unsqueeze(2).broadcast_to([P, G, 3, 8]),
                    op=Alu.mult)

                # ---- scatter-add segment sum (chunks of 2 groups, overlapped)
                if (g + 1) % 2 == 0:
                    c = (g + 1) // 2 - 1
                    tq = 2 * G
                    nc.gpsimd.dma_scatter_add(
                        out_ap=out_d[:, 0:32],
                        in_ap=eo[:, c * tq:(c + 1) * tq, :],
                        idxs_ap=sidx[:, c * (tq * P // 16):(c + 1) * (tq * P // 16)],
                        num_idxs=tq * P,
                        num_idxs_reg=tq * P,
                        elem_size=32,
                        elem_step=64,
                    )
    nc.compile()
    return nc


def _get_nc(key):
    if key not in _KERNEL_CACHE:
        _KERNEL_CACHE[key] = _build(*key)
    return _KERNEL_CACHE[key]


# --------------------------------------------------------------------------
# Entry point
# --------------------------------------------------------------------------

def kernel(x, edge_src, edge_dst, edge_vec, emb, norm, num_nodes,
           fc1_w1, fc1_w2, fc2_w1, fc2_w2, _trace=False):
    x = np.asarray(x, np.float32)
    edge_src = np.asarray(edge_src).astype(np.int64)
    edge_dst = np.asarray(edge_dst).astype(np.int64)
    edge_vec = np.asarray(edge_vec, np.float32)
    emb = np.asarray(emb, np.float32)
    norm = np.asarray(norm, np.float32)
    fc1_w1 = np.asarray(fc1_w1, np.float32)
    fc1_w2 = np.asarray(fc1_w2, np.float32)
    fc2_w1 = np.asarray(fc2_w1, np.float32)
    fc2_w2 = np.asarray(fc2_w2, np.float32)
    N = x.shape[0]
    assert int(num_nodes) == N

    in_maps, N, npc, wpc, t_w, nt, ep = _host_prep(
        x, edge_src, edge_dst, edge_vec, emb, norm,
        fc1_w1, fc1_w2, fc2_w1, fc2_w2)
    nc = _get_nc((N, npc, wpc, t_w, nt, ep))
    res = run_bass_kernel_spmd(nc, in_maps, core_ids=list(range(NCORES)),
                               trace=_trace)
    acc = np.zeros((N, 64), np.float64)
    for m in range(NCORES):
        acc += res.results[m]["out"].astype(np.float64)
    got = acc[:, 0:32].astype(np.float32)
    # columns 8:32 are (k, w)-major on device; reference wants (w, k)
    out = np.empty_like(got)
    out[:, 0:8] = got[:, 0:8]
    out[:, 8:32] = got[:, 8:32].reshape(N, 3, 8).transpose(0, 2, 1).reshape(N, 24)
    if _trace:
        return out, res
    return out


# revision 19
# speedup vs baseline: 3.5112x; 1.1216x over previous
"""Trainium2 Bass kernel for nn_Eq_NLMP_18013092840057 (gnn_message_passing).

Strategy (v2):
  * Host: equal edge split across 8 cores (20000 edges + pad to 20480).
    Host pre-gathers x[src]/x[dst], precomputes the radial-MLP hidden
    layers h1/h2 (10->16), r-hat, vdot1, and lays everything out
    edge-major bf16 ([128, nt, F], edge i at partition i%128, tile
    i//128).  The fc2 weight matrices are permuted to (w-outer, u-inner)
    blocks and scaled so all path norms fold in.
  * Device (per core): PE generates per-edge TP weights in bf16
    (h-tile [16,128] x fcw [16,896] per tile of 128 edges); ACT
    evacuates PSUM->SBUF bf16; DVE + GPSIMD split the equivariant
    tensor-product mults and reduction trees (all bf16, 2x packed mode);
    tanh gating on ACT; a single dma_scatter_add accumulates edge
    outputs into a per-core [N, 64] HBM buffer.
  * Host: sums the 8 per-core partials and reorders vector columns.
"""
import sys
import numpy as np

try:
    import concourse.bass as bass  # noqa: F401
except Exception:  # pragma: no cover
    sys.path.insert(0, "/opt/trn_rl_repo")

import concourse.bass as bass
import concourse.bacc as bacc
import concourse.tile as tile
from concourse import mybir
from concourse.bass_utils import run_bass_kernel_spmd

SQRT3 = np.float32(3.0 ** 0.5)
P = 128
NCORES = 8
G = 16               # tiles per group
dt = mybir.dt
Alu = mybir.AluOpType
Act = mybir.ActivationFunctionType

_KERNEL_CACHE = {}


# --------------------------------------------------------------------------
# Host-side preparation
# --------------------------------------------------------------------------

def _prep_fcw(fc1_w2, fc2_w2):
    """Permute + scale the 16->512 / 16->384 weight mats into one [16, 896]
    block layout: [ssvv(256) | sv(128) | vs(128) | AB(256) | Csv(64) | Cvs(64)],
    every block (w-outer, u-inner)."""
    a1 = np.float32(1.0 / np.sqrt(32.0))
    a2 = np.float32(1.0 / np.sqrt(16.0))
    s = np.float32(1.0 / np.sqrt(16.0))       # MLP second-layer 1/sqrt(fan_in)
    f1 = (fc1_w2 * s).astype(np.float32)      # [16, 512]
    f2 = (fc2_w2 * s).astype(np.float32)      # [16, 384]
    cols = np.zeros((16, 896), np.float32)
    # ssvv block: cols 0:256, c = w*32 + u; u<16 -> Wss (f1[:, u*8+w]) * a1,
    # u>=16 -> Wvv (f1[:, 128 + (u-16)*8 + w]) * a1
    for w in range(8):
        for u in range(32):
            src = u * 8 + w if u < 16 else 128 + (u - 16) * 8 + w
            cols[:, w * 32 + u] = f1[:, src] * a1
    # sv block: cols 256:384, c = 256 + w*16 + u ; f1[:, 256 + u*8+w] * a1*sqrt3
    for w in range(8):
        for u in range(16):
            cols[:, 256 + w * 16 + u] = f1[:, 256 + u * 8 + w] * (a1 * SQRT3)
    # vs block: cols 384:512 ; f1[:, 384 + u*8+w] * a1
    for w in range(8):
        for u in range(16):
            cols[:, 384 + w * 16 + u] = f1[:, 384 + u * 8 + w] * a1
    # AB block: cols 512:768. q=0 (A): u<8 Ass f2[:, u*8+w], u>=8 Avv
    # f2[:, 64 + (u-8)*8 + w]; q=1 (B): Bss 128+, Bvv 192+. all * a2
    for q in range(2):
        for w in range(8):
            for u in range(16):
                base = (0 if u < 8 else 64) + q * 128
                src = base + (u % 8) * 8 + w
                cols[:, 512 + q * 128 + w * 16 + u] = f2[:, src] * a2
    # Csv: cols 768:832 ; f2[:, 256 + u*8+w] * a2*sqrt3
    for w in range(8):
        for u in range(8):
            cols[:, 768 + w * 8 + u] = f2[:, 256 + u * 8 + w] * (a2 * SQRT3)
    # Cvs: cols 832:896 ; f2[:, 320 + u*8+w] * a2
    for w in range(8):
        for u in range(8):
            cols[:, 832 + w * 8 + u] = f2[:, 320 + u * 8 + w] * a2
    return cols


def _host_prep(x, edge_src, edge_dst, edge_vec, emb, norm,
               fc1_w1, fc1_w2, fc2_w1, fc2_w2):
    import ml_dtypes
    bf16 = ml_dtypes.bfloat16
    N = x.shape[0]
    E = edge_src.shape[0]
    epc = E // NCORES                          # edges per core (true)
    nt = ((epc + G * P - 1) // (G * P)) * G    # tiles per core (mult of G)
    ep = nt * P                                # padded edges per core

    fcw = _prep_fcw(fc1_w2, fc2_w2)
    h1 = np.maximum(emb @ fc1_w1 / np.sqrt(np.float32(10.0)), 0.0).astype(np.float32)
    h2 = np.maximum(emb @ fc2_w1 / np.sqrt(np.float32(10.0)), 0.0).astype(np.float32)
    rhat = (edge_vec / np.linalg.norm(edge_vec, axis=1, keepdims=True)).astype(np.float32)

    xs = x[edge_src]                           # [E, 32]
    xd = x[edge_dst]
    s1 = np.concatenate([xs[:, :8], xd[:, :8]], axis=1)          # [E,16]
    v1 = np.concatenate([xs[:, 8:].reshape(E, 8, 3),
                         xd[:, 8:].reshape(E, 8, 3)], axis=1)    # [E,16,3]
    vdot1 = np.einsum('euk,ek->eu', v1, rhat)                    # [E,16]
    f1 = np.concatenate([s1, vdot1], axis=1)                     # [E,32]
    v1k = np.ascontiguousarray(v1.transpose(0, 2, 1))            # [E,3,16]
    rr8 = np.repeat(rhat[:, :, None], 8, axis=2)                 # [E,3,8]
    n8 = np.repeat(norm[:, None], 8, axis=1)                     # [E,8]

    def interleave(arr, m):
        # core m's slice -> [P, nt, F]; edge i at [i%P, i//P]
        F = int(np.prod(arr.shape[1:])) if arr.ndim > 1 else 1
        a = arr[m * epc:(m + 1) * epc].reshape(epc, F)
        a = np.concatenate([a, np.zeros((ep - epc, F), a.dtype)], axis=0)
        return np.ascontiguousarray(a.reshape(nt, P, F).transpose(1, 0, 2)).astype(bf16)

    in_maps = []
    for m in range(NCORES):
        h1t = np.zeros((16, ep), np.float32)
        h2t = np.zeros((16, ep), np.float32)
        h1t[:, :epc] = h1[m * epc:(m + 1) * epc].T
        h2t[:, :epc] = h2[m * epc:(m + 1) * epc].T

        dst_c = edge_dst[m * epc:(m + 1) * epc].astype(np.int16)
        idx = np.zeros((P, ep // 16), np.int16)
        ii = np.arange(epc)
        idx[ii % 16, ii // 16] = dst_c

        in_maps.append({
            "fcw": fcw.astype(bf16),
            "h1t": h1t.astype(bf16),
            "h2t": h2t.astype(bf16),
            "f1": interleave(f1, m),
            "v1k": interleave(v1k, m),
            "rr8": interleave(rr8, m),
            "n8": interleave(n8, m),
            "sidx": idx,
            "zero32": np.zeros((N, 32), np.float32),
        })
    return in_maps, N, 0, 0, 0, nt, ep


# --------------------------------------------------------------------------
# Bass program
# --------------------------------------------------------------------------

def _build(N, npc, wpc, t_w, nt, ep):
    nc = bacc.Bacc("TRN2", target_bir_lowering=False)
    f32, bf16 = dt.float32, dt.bfloat16
    ng = nt // G

    fcw_d = nc.dram_tensor("fcw", [16, 896], bf16, kind="ExternalInput")
    h1t_d = nc.dram_tensor("h1t", [16, ep], bf16, kind="ExternalInput")
    h2t_d = nc.dram_tensor("h2t", [16, ep], bf16, kind="ExternalInput")
    f1_d = nc.dram_tensor("f1", [P, nt, 32], bf16, kind="ExternalInput")
    v1k_d = nc.dram_tensor("v1k", [P, nt, 48], bf16, kind="ExternalInput")
    rr8_d = nc.dram_tensor("rr8", [P, nt, 24], bf16, kind="ExternalInput")
    n8_d = nc.dram_tensor("n8", [P, nt, 8], bf16, kind="ExternalInput")
    sidx_d = nc.dram_tensor("sidx", [P, ep // 16], dt.int16, kind="ExternalInput")
    zero_d = nc.dram_tensor("zero32", [N, 32], f32, kind="ExternalInput")
    out_d = nc.dram_tensor("out", [N, 64], f32, kind="ExternalOutput")

    with tile.TileContext(nc) as tc:
        with tc.tile_pool(name="const", bufs=1) as cpool, \
             tc.tile_pool(name="io", bufs=2) as io, \
             tc.tile_pool(name="wsb", bufs=2) as wsb, \
             tc.tile_pool(name="mm", bufs=2) as mm, \
             tc.tile_pool(name="sm", bufs=2) as sm, \
             tc.tile_pool(name="eop", bufs=1) as eop, \
             tc.tile_pool(name="wps", bufs=2, space="PSUM") as wps:

            fcw = cpool.tile([16, 896], bf16)
            nc.sync.dma_start(fcw[:], fcw_d[:, :])
            sidx = cpool.tile([P, ep // 16], dt.int16)
            nc.sync.dma_start(sidx[:], sidx_d[:, :])
            eo = None

            # zero the output accumulator up front; the first scatter-add
            # fires ~2 full compute groups (tens of us) later
            nc.sync.dma_start(out_d[:, 0:32], zero_d[:, :])

            for g in range(ng):
                tb = g * G
                if g % 2 == 0:
                    # chunk-local edge-output staging, double buffered so the
                    # scatter's read doesn't block the next chunk's writes
                    eo = eop.tile([P, 2 * G, 32], f32, tag="eo")
                eb = (g % 2) * G

                h1g = io.tile([16, G * P], bf16, tag="h1g")
                h2g = io.tile([16, G * P], bf16, tag="h2g")
                f1g = io.tile([P, G, 32], bf16, tag="f1")
                v1g = io.tile([P, G, 3, 16], bf16, tag="v1")
                rr8g = io.tile([P, G, 3, 8], bf16, tag="rr8")
                n8g = io.tile([P, G, 8], bf16, tag="n8")
                nc.sync.dma_start(h1g[:], h1t_d[:, tb * P:(tb + G) * P])
                nc.sync.dma_start(h2g[:], h2t_d[:, tb * P:(tb + G) * P])
                nc.sync.dma_start(f1g[:], f1_d[:, tb:tb + G, :].rearrange(
                    "p t f -> p t f"))
                nc.sync.dma_start(v1g[:], v1k_d[:, tb:tb + G, :].rearrange(
                    "p t (k u) -> p t k u", k=3))
                nc.sync.dma_start(rr8g[:], rr8_d[:, tb:tb + G, :].rearrange(
                    "p t (k w) -> p t k w", k=3))
                nc.sync.dma_start(n8g[:], n8_d[:, tb:tb + G, :])

                # ---- PE weight-gen + ACT evacuation ----------------------
                W = wsb.tile([P, G, 896], bf16, tag="W")
                for q in range(G // 2):
                    wp = wps.tile([P, 2, 1024], f32, tag="wp")
                    for i in range(2):
                        t = 2 * q + i
                        nc.tensor.matmul(out=wp[:, i, 0:512],
                                         lhsT=h1g[:, t * P:(t + 1) * P],
                                         rhs=fcw[:, 0:512], start=True, stop=True)
                        nc.tensor.matmul(out=wp[:, i, 512:896],
                                         lhsT=h2g[:, t * P:(t + 1) * P],
                                         rhs=fcw[:, 512:896], start=True, stop=True)
                    nc.scalar.copy(W[:, 2 * q:2 * q + 2, :], wp[:, :, 0:896])

                # ---- TP1 ss+vv path (DVE) --------------------------------
                m32 = mm.tile([P, G, 8, 32], bf16, tag="m32")
                nc.vector.tensor_tensor(
                    out=m32[:],
                    in0=W[:, :, 0:256].rearrange("p g (w u) -> p g w u", w=8),
                    in1=f1g[:].unsqueeze(2).broadcast_to([P, G, 8, 32]),
                    op=Alu.mult)
                m32v = m32[:].rearrange("p g w u -> p (g w) u")
                for k in (16, 8, 4, 2):
                    nc.vector.tensor_tensor(out=m32v[:, :, 0:k], in0=m32v[:, :, 0:k],
                                            in1=m32v[:, :, k:2 * k], op=Alu.add)
                f2 = sm.tile([P, G, 16], bf16, tag="f2")
                nc.vector.tensor_tensor(out=f2[:, :, 0:8], in0=m32[:, :, :, 0],
                                        in1=m32[:, :, :, 1], op=Alu.add)

                # ---- TP1 sv path (Pool) ----------------------------------
                msv = mm.tile([P, G, 8, 16], bf16, tag="msv")
                nc.gpsimd.tensor_tensor(
                    out=msv[:],
                    in0=W[:, :, 256:384].rearrange("p g (w u) -> p g w u", w=8),
                    in1=f1g[:, :, 0:16].unsqueeze(2).broadcast_to([P, G, 8, 16]),
                    op=Alu.mult)
                msvv = msv[:].rearrange("p g w u -> p (g w) u")
                for k in (8, 4, 2):
                    nc.gpsimd.tensor_tensor(out=msvv[:, :, 0:k], in0=msvv[:, :, 0:k],
                                            in1=msvv[:, :, k:2 * k], op=Alu.add)
                ssv = sm.tile([P, G, 8], bf16, tag="ssv")
                nc.gpsimd.tensor_tensor(out=ssv[:], in0=msv[:, :, :, 0],
                                        in1=msv[:, :, :, 1], op=Alu.add)

                # ---- TP1 vs path (Pool) ----------------------------------
                m16p = mm.tile([P, G, 3, 8, 16], bf16, tag="m16p")
                for k in range(3):
                    nc.gpsimd.tensor_tensor(
                        out=m16p[:, :, k, :, :],
                        in0=W[:, :, 384:512].rearrange("p g (w u) -> p g w u", w=8),
                        in1=v1g[:, :, k, :].unsqueeze(2).broadcast_to([P, G, 8, 16]),
                        op=Alu.mult)
                m16pv = m16p[:].rearrange("p g k w u -> p (g k w) u")
                for k in (8, 4, 2):
                    nc.gpsimd.tensor_tensor(out=m16pv[:, :, 0:k], in0=m16pv[:, :, 0:k],
                                            in1=m16pv[:, :, k:2 * k], op=Alu.add)
                vts = sm.tile([P, G, 3, 8], bf16, tag="vts")
                nc.gpsimd.tensor_tensor(out=vts[:], in0=m16p[:, :, :, :, 0],
                                        in1=m16p[:, :, :, :, 1], op=Alu.add)

                # ---- v_t, vdot2 (DVE) ------------------------------------
                v_t = sm.tile([P, G, 3, 8], bf16, tag="v_t")
                nc.vector.tensor_tensor(
                    out=v_t[:],
                    in0=ssv[:].unsqueeze(2).broadcast_to([P, G, 3, 8]),
                    in1=rr8g[:], op=Alu.mult)
                nc.vector.tensor_tensor(out=v_t[:], in0=v_t[:], in1=vts[:], op=Alu.add)
                vd3 = sm.tile([P, G, 3, 8], bf16, tag="vd3")
                nc.vector.tensor_tensor(out=vd3[:], in0=v_t[:], in1=rr8g[:], op=Alu.mult)
                vd2 = sm.tile([P, G, 8], bf16, tag="vd2")
                nc.vector.tensor_tensor(out=vd2[:], in0=vd3[:, :, 0, :],
                                        in1=vd3[:, :, 1, :], op=Alu.add)
                nc.vector.tensor_tensor(out=f2[:, :, 8:16], in0=vd2[:],
                                        in1=vd3[:, :, 2, :], op=Alu.add)

                # ---- TP2 A+B paths (DVE) ---------------------------------
                mab = mm.tile([P, G, 16, 16], bf16, tag="mab")
                nc.vector.tensor_tensor(
                    out=mab[:],
                    in0=W[:, :, 512:768].rearrange("p g (w u) -> p g w u", w=16),
                    in1=f2[:].unsqueeze(2).broadcast_to([P, G, 16, 16]),
                    op=Alu.mult)
                mabv = mab[:].rearrange("p g w u -> p (g w) u")
                for k in (8, 4, 2):
                    nc.vector.tensor_tensor(out=mabv[:, :, 0:k], in0=mabv[:, :, 0:k],
                                            in1=mabv[:, :, k:2 * k], op=Alu.add)
                sg = sm.tile([P, G, 2, 8], bf16, tag="sg")
                nc.vector.tensor_tensor(
                    out=sg[:], in0=mab[:, :, :, 0].rearrange("p g (q w) -> p g q w", q=2),
                    in1=mab[:, :, :, 1].rearrange("p g (q w) -> p g q w", q=2),
                    op=Alu.add)

                # ---- TP2 Csv path (DVE) ----------------------------------
                mcs = mm.tile([P, G, 8, 8], bf16, tag="mcs")
                nc.vector.tensor_tensor(
                    out=mcs[:],
                    in0=W[:, :, 768:832].rearrange("p g (w u) -> p g w u", w=8),
                    in1=f2[:, :, 0:8].unsqueeze(2).broadcast_to([P, G, 8, 8]),
                    op=Alu.mult)
                mcsv = mcs[:].rearrange("p g w u -> p (g w) u")
                for k in (4, 2):
                    nc.vector.tensor_tensor(out=mcsv[:, :, 0:k], in0=mcsv[:, :, 0:k],
                                            in1=mcsv[:, :, k:2 * k], op=Alu.add)
                scs = sm.tile([P, G, 8], bf16, tag="scs")
                nc.vector.tensor_tensor(out=scs[:], in0=mcs[:, :, :, 0],
                                        in1=mcs[:, :, :, 1], op=Alu.add)

                # ---- TP2 Cvs path (DVE) ----------------------------------
                mcv = mm.tile([P, G, 3, 8, 8], bf16, tag="mcv")
                for k in range(3):
                    nc.vector.tensor_tensor(
                        out=mcv[:, :, k, :, :],
                        in0=W[:, :, 832:896].rearrange("p g (w u) -> p g w u", w=8),
                        in1=v_t[:, :, k, :].unsqueeze(2).broadcast_to([P, G, 8, 8]),
                        op=Alu.mult)
                mcvv = mcv[:].rearrange("p g k w u -> p (g k w) u")
                for k in (4, 2):
                    nc.vector.tensor_tensor(out=mcvv[:, :, 0:k], in0=mcvv[:, :, 0:k],
                                            in1=mcvv[:, :, k:2 * k], op=Alu.add)
                vcs = sm.tile([P, G, 3, 8], bf16, tag="vcs")
                nc.vector.tensor_tensor(out=vcs[:], in0=mcv[:, :, :, :, 0],
                                        in1=mcv[:, :, :, :, 1], op=Alu.add)

                # ---- gate + pack ----------------------------------------
                vecs = sm.tile([P, G, 3, 8], bf16, tag="vecs")
                nc.vector.tensor_tensor(
                    out=vecs[:],
                    in0=scs[:].unsqueeze(2).broadcast_to([P, G, 3, 8]),
                    in1=rr8g[:], op=Alu.mult)
                nc.vector.tensor_tensor(out=vecs[:], in0=vecs[:], in1=vcs[:], op=Alu.add)
                tsg = sm.tile([P, G, 2, 8], bf16, tag="tsg")
                nc.scalar.activation(tsg[:], sg[:], Act.Tanh)
                tgn = sm.tile([P, G, 8], bf16, tag="tgn")
                nc.vector.tensor_tensor(out=tgn[:], in0=tsg[:, :, 1, :],
                                        in1=n8g[:], op=Alu.mult)
                nc.vector.tensor_tensor(out=eo[:, eb:eb + G, 0:8],
                                        in0=tsg[:, :, 0, :], in1=n8g[:], op=Alu.mult)
                nc.vector.tensor_tensor(
                    out=eo[:, eb:eb + G, 8:32].rearrange("p g (k w) -> p g k w", k=3),
                    in0=vecs[:],
                    in1=tgn[:].unsqueeze(2).broadcast_to([P, G, 3, 8]),
                    op=Alu.mult)

                # ---- scatter-add segment sum (chunks of 2 groups, overlapped)
                if (g + 1) % 2 == 0:
                    c = (g + 1) // 2 - 1
                    tq = 2 * G
                    nc.gpsimd.dma_scatter_add(
                        out_ap=out_d[:, 0:32],
                        in_ap=eo[:],
                        idxs_ap=sidx[:, c * (tq * P // 16):(c + 1) * (tq * P // 16)],
                        num_idxs=tq * P,
                        num_idxs_reg=tq * P,
                        elem_size=32,
                        elem_step=64,
                    )
    nc.compile()
    return nc


def _get_nc(key):
    if key not in _KERNEL_CACHE:
        _KERNEL_CACHE[key] = _build(*key)
    return _KERNEL_CACHE[key]


# --------------------------------------------------------------------------
# Entry point
# --------------------------------------------------------------------------

def kernel(x, edge_src, edge_dst, edge_vec, emb, norm, num_nodes,
           fc1_w1, fc1_w2, fc2_w1, fc2_w2, _trace=False):
    x = np.asarray(x, np.float32)
    edge_src = np.asarray(edge_src).astype(np.int64)
    edge_dst = np.asarray(edge_dst).astype(np.int64)
    edge_vec = np.asarray(edge_vec, np.float32)
    emb = np.asarray(emb, np.float32)
    norm = np.asarray(norm, np.float32)
    fc1_w1 = np.asarray(fc1_w1, np.float32)
    fc1_w2 = np.asarray(fc1_w2, np.float32)
    fc2_w1 = np.asarray(fc2_w1, np.float32)
    fc2_w2 = np.asarray(fc2_w2, np.float32)
    N = x.shape[0]
    assert int(num_nodes) == N

    in_maps, N, npc, wpc, t_w, nt, ep = _host_prep(
        x, edge_src, edge_dst, edge_vec, emb, norm,
        fc1_w1, fc1_w2, fc2_w1, fc2_w2)
    nc = _get_nc((N, npc, wpc, t_w, nt, ep))
    res = run_bass_kernel_spmd(nc, in_maps, core_ids=list(range(NCORES)),
                               trace=_trace)
    acc = np.zeros((N, 64), np.float64)
    for m in range(NCORES):
        acc += res.results[m]["out"].astype(np.float64)
    got = acc[:, 0:32].astype(np.float32)
    # columns 8:32 are (k, w)-major on device; reference wants (w, k)
    out = np.empty_like(got)
    out[:, 0:8] = got[:, 0:8]
    out[:, 8:32] = got[:, 8:32].reshape(N, 3, 8).transpose(0, 2, 1).reshape(N, 24)
    if _trace:
        return out, res
    return out
